# revision 1
# baseline (speedup 1.0000x reference)
"""Biased multi-head attention Trainium2 kernel (Bass/Tile), 8-way data-parallel over batch.

Reference computation (per batch b):
  q = (nd @ Wq + bq) * 8 ; k = nd @ Wk + bk ; v = nd @ Wv + bv      (8 heads, d=64)
  S[h] = Q_h K_h^T + bias[..,h] ; S[mask==1] = -inf
  A = softmax(S, -1) * mul[..,h]
  out = concat_h(A_h V_h) @ Wo + bo

Device mapping (per core, 2 batches):
  - ndT via PE transposes; QT/KT = W^T @ ndT (f on partitions -> per-partition bias adds)
  - V = nd @ Wv (+bv via a K=1 ones-row matmul into the same PSUM accumulation group)
  - scores matmul K=64 per (head, i-chunk); bias-add fused with row-max via
    tensor_tensor_reduce (out=-(S+bias), accum=min -> -rowmax)
  - exp on ACT with per-partition bias=-rowmax, scale=-1; E in bf16
  - mask applied post-exp as E*(1-mask) fused with Z=sum via a second TTR
  - A = (E * (1/Z)) * mul via scalar_tensor_tensor
  - A transposed per 128x128 block on PE; AV matmul col-packs 2 heads per PSUM bank
  - final projection from OT (bf16) @ Wo (bf16) + bo ones-row matmul
"""

import os
import sys

import numpy as np

try:
    import concourse  # noqa: F401
except ImportError:
    sys.path.insert(0, "/opt/trn_rl_repo")

import ml_dtypes
from concourse import bass, mybir
from concourse.bass_utils import run_bass_kernel_spmd
from concourse.tile import TileContext

B, N, F, H, D = 16, 512, 512, 8, 64
NCORES = 8
BPC = B // NCORES  # batches per core
IC = N // 128      # 128-row chunks per sequence

f32 = mybir.dt.float32
bf16 = mybir.dt.bfloat16
i32 = mybir.dt.int32
AF = mybir.ActivationFunctionType
OP = mybir.AluOpType


def _split_multiwaits(nc: bass.Bass) -> bass.Bass:
    """Walrus codegen only accepts one sync-wait per ISA instruction; hoist
    extra waits into single-wait NoOps on the same engine right before."""
    for fn in nc.m.functions:
        for blk in fn.blocks:
            new = []
            for inst in blk.instructions:
                si = getattr(inst, "sync_info", None)
                ow = list(si.on_wait) if (si is not None and si.on_wait) else []
                if len(ow) > 1:
                    for j, w in enumerate(ow[:-1]):
                        new.append(mybir.InstNoOp(
                            name=f"{inst.name}-wsplit{j}",
                            engine=inst.engine,
                            ins=[], outs=[],
                            sync_info=mybir.SyncInfo(on_wait=[w], on_update=[]),
                        ))
                    si.on_wait = ow[-1:]
                    inst.sync_info = si
                new.append(inst)
            blk.instructions[:] = new
    return nc


def build_nc(split: bool = True) -> bass.Bass:
    nc = bass.Bass()

    nd_d = nc.declare_dram_parameter("nd", [BPC, N, F], f32, isOutput=False)
    # bias with mask pre-merged (-1e30 at masked positions), bf16, layout [N, N, H]
    bias_d = nc.declare_dram_parameter("bias", [BPC, N, N, H], bf16, isOutput=False)
    # attn_mul transposed to [H, N, N], bf16
    mul_d = nc.declare_dram_parameter("mul", [BPC, H, N, N], bf16, isOutput=False)
    wq_d = nc.declare_dram_parameter("wq", [F, F], f32, isOutput=False)  # pre-scaled x8
    wk_d = nc.declare_dram_parameter("wk", [F, F], f32, isOutput=False)
    wv_d = nc.declare_dram_parameter("wv", [F, F], f32, isOutput=False)
    wo_d = nc.declare_dram_parameter("wo", [F, F], bf16, isOutput=False)
    bq_d = nc.declare_dram_parameter("bq", [F, 1], f32, isOutput=False)  # pre-scaled x8
    bk_d = nc.declare_dram_parameter("bk", [F, 1], f32, isOutput=False)
    # [bv(512) | bo(512) | ones(128)]
    brow_d = nc.declare_dram_parameter("brow", [1, F + F + 128], f32, isOutput=False)
    idf_d = nc.declare_dram_parameter("idf", [128, 128], f32, isOutput=False)
    idb_d = nc.declare_dram_parameter("idb", [128, 128], bf16, isOutput=False)
    out_d = nc.declare_dram_parameter("out", [BPC, N, F], f32, isOutput=True)

    with (
        TileContext(nc) as tc,
        tc.tile_pool(name="cpool", bufs=1) as cpool,
        tc.tile_pool(name="bpool", bufs=1) as bpool,
        tc.tile_pool(name="spool", bufs=2) as spool,
        tc.tile_pool(name="wpool", bufs=2) as wpool,
        tc.tile_pool(name="ps_mm", bufs=2, space="PSUM") as ps_mm,
        tc.tile_pool(name="ps_t", bufs=2, space="PSUM") as ps_t,
        tc.tile_pool(name="ps_o", bufs=2, space="PSUM") as ps_o,
    ):
        # ---- constants / weights ----
        wq_sb = cpool.tile([128, 4 * F], f32, name="wq_sb")
        wk_sb = cpool.tile([128, 4 * F], f32, name="wk_sb")
        wv_sb = cpool.tile([128, 4 * F], f32, name="wv_sb")
        wo_sb = cpool.tile([128, 4 * F], bf16, name="wo_sb")
        for cc in range(4):
            nc.sync.dma_start(wq_sb[:, cc * F:(cc + 1) * F], wq_d[cc * 128:(cc + 1) * 128, :])
            nc.sync.dma_start(wk_sb[:, cc * F:(cc + 1) * F], wk_d[cc * 128:(cc + 1) * 128, :])
        # wv/wo DMAs are issued inside the first batch body, after the
        # latency-critical nd/biasm loads
        bq_sb = cpool.tile([128, 4], f32, name="bq_sb")
        bk_sb = cpool.tile([128, 4], f32, name="bk_sb")
        for ft in range(4):
            nc.sync.dma_start(bq_sb[:, ft:ft + 1], bq_d[ft * 128:(ft + 1) * 128, :])
            nc.sync.dma_start(bk_sb[:, ft:ft + 1], bk_d[ft * 128:(ft + 1) * 128, :])
        brow_sb = cpool.tile([1, F + F + 128], f32, name="brow_sb")
        nc.sync.dma_start(brow_sb[:], brow_d[:, :])
        bv_row = brow_sb[:, 0:F]
        bo_row = brow_sb[:, F:2 * F]
        ones_row = brow_sb[:, 2 * F:2 * F + 128]
        idf_sb = cpool.tile([128, 128], f32, name="idf_sb")
        nc.sync.dma_start(idf_sb[:], idf_d[:, :])
        idb_sb = cpool.tile([128, 128], bf16, name="idb_sb")
        nc.sync.dma_start(idb_sb[:], idb_d[:, :])

        def batch_inputs(b):
            st = {}
            nd_sb = bpool.tile([128, IC * F], f32, name="nd_sb", tag="nd", bufs=2)
            for ic in range(IC):
                nc.sync.dma_start(nd_sb[:, ic * F:(ic + 1) * F],
                                  nd_d[b, ic * 128:(ic + 1) * 128, :])
            st["nd"] = nd_sb
            st["biasm"] = []
            for ic in range(IC):
                biasm_sb = spool.tile([128, N * H], bf16, name="biasm_sb",
                                      tag="biasm", bufs=5)
                nc.sync.dma_start(
                    biasm_sb[:],
                    bias_d[b, ic * 128:(ic + 1) * 128].rearrange("p j h -> p (j h)"))
                # view ordered [p, h, j] so a pair slice matches PSUM (u, j) layout
                st["biasm"].append(biasm_sb.rearrange("p (j h) -> p h j", h=H))
            if b == 0:
                for cc in range(4):
                    nc.sync.dma_start(wv_sb[:, cc * F:(cc + 1) * F],
                                      wv_d[cc * 128:(cc + 1) * 128, :])
                    nc.sync.dma_start(wo_sb[:, cc * F:(cc + 1) * F],
                                      wo_d[cc * 128:(cc + 1) * 128, :])
            return st

        def ndt_stage(st):
            nd_sb = st["nd"]
            ndt_sb = bpool.tile([128, 4 * N], f32, name="ndt_sb", tag="ndt", bufs=1)
            for cc in range(4):
                ps = ps_mm.tile([128, 512], f32, name="ps_nt", tag="mm")
                for ic in range(IC):
                    nc.tensor.transpose(
                        ps[:, ic * 128:(ic + 1) * 128],
                        nd_sb[:, ic * F + cc * 128: ic * F + cc * 128 + 128],
                        idf_sb[:],
                    )
                nc.scalar.copy(ndt_sb[:, cc * N:(cc + 1) * N], ps[:])
            st["ndt"] = ndt_sb
            st["qt"] = bpool.tile([128, 4 * N], f32, name="qt_sb", tag="qt", bufs=1)
            st["kt"] = bpool.tile([128, 4 * N], f32, name="kt_sb", tag="kt", bufs=1)
            st["v"] = bpool.tile([128, 4 * F], bf16, name="v_sb", tag="v", bufs=2)
            st["ot"] = bpool.tile([128, 4 * N], bf16, name="ot_sb", tag="ot", bufs=1)

        def qk_proj(st, ft):
            ndt_sb, qt_sb, kt_sb = st["ndt"], st["qt"], st["kt"]
            psq = ps_mm.tile([128, 512], f32, name="ps_q", tag="mm")
            for cc in range(4):
                nc.tensor.matmul(
                    psq[:],
                    lhsT=wq_sb[:, cc * F + ft * 128: cc * F + ft * 128 + 128],
                    rhs=ndt_sb[:, cc * N:(cc + 1) * N],
                    start=(cc == 0), stop=(cc == 3),
                )
            nc.scalar.activation(qt_sb[:, ft * N:(ft + 1) * N], psq[:],
                                 AF.Identity, bias=bq_sb[:, ft:ft + 1], scale=1.0)
            psk = ps_mm.tile([128, 512], f32, name="ps_k", tag="mm")
            for cc in range(4):
                nc.tensor.matmul(
                    psk[:],
                    lhsT=wk_sb[:, cc * F + ft * 128: cc * F + ft * 128 + 128],
                    rhs=ndt_sb[:, cc * N:(cc + 1) * N],
                    start=(cc == 0), stop=(cc == 3),
                )
            nc.scalar.activation(kt_sb[:, ft * N:(ft + 1) * N], psk[:],
                                 AF.Identity, bias=bk_sb[:, ft:ft + 1], scale=1.0)

        def v_proj(st, jc):
            ndt_sb, v_sb = st["ndt"], st["v"]
            psv = ps_mm.tile([128, 512], f32, name="ps_v", tag="mm")
            for cc in range(4):
                nc.tensor.matmul(
                    psv[:],
                    lhsT=ndt_sb[:, cc * N + jc * 128: cc * N + jc * 128 + 128],
                    rhs=wv_sb[:, cc * F:(cc + 1) * F],
                    start=(cc == 0), stop=False,
                )
            nc.tensor.matmul(psv[:], lhsT=ones_row, rhs=bv_row,
                             start=False, stop=True)
            nc.scalar.copy(v_sb[:, jc * F:(jc + 1) * F], psv[:])

        def s_stage(b, st, t, pm_sb):
            """Scores + softmax for head pair t -> pm_sb. DVE stream is
            software-pipelined: recip/STT of chunk ic are emitted after
            TT/reduce of chunk ic+1 so the in-order DVE never stalls on ACT."""
            qt_sb, kt_sb, biasm_tiles = st["qt"], st["kt"], st["biasm"]
            h0 = 2 * t
            ft = t
            muls = []
            for u in range(2):
                mul_sb = spool.tile([128, IC * N], bf16, name="mul_sb",
                                    tag="mul", bufs=4)
                nc.sync.dma_start(
                    mul_sb.rearrange("p (ic j) -> p ic j", ic=IC),
                    mul_d[b, h0 + u].rearrange("(ic p) j -> p ic j", p=128))
                muls.append(mul_sb)

            pend = [None]

            def flush_pend():
                if pend[0] is None:
                    return
                ic, e_sbs, z2 = pend[0]
                rz2 = wpool.tile([128, 2], f32, name="rz2", bufs=4)
                nc.vector.reciprocal(rz2[:], z2[:])
                for u in range(2):
                    nc.vector.scalar_tensor_tensor(
                        out=pm_sb[:, (u * IC + ic) * N:(u * IC + ic + 1) * N],
                        in0=e_sbs[u][:], scalar=rz2[:, u:u + 1],
                        in1=muls[u][:, ic * N:(ic + 1) * N],
                        op0=OP.mult, op1=OP.mult,
                    )
                pend[0] = None

            for ic in range(IC):
                sp = ps_mm.tile([128, 2 * N], f32, name="sp_ps", tag="mm")
                for u in range(2):
                    nc.tensor.matmul(
                        sp[:, u * N:(u + 1) * N],
                        lhsT=qt_sb[u * 64:u * 64 + 64,
                                   ft * N + ic * 128: ft * N + ic * 128 + 128],
                        rhs=kt_sb[u * 64:u * 64 + 64, ft * N:(ft + 1) * N],
                        start=True, stop=True,
                    )
                sp_v = sp.rearrange("p (u j) -> p u j", u=2)
                # bias-add lands in SBUF so the PSUM pair-bank frees early
                s2_sb = wpool.tile([128, 2 * N], f32, name="s2_sb",
                                   tag="s2", bufs=3)
                s2_v = s2_sb.rearrange("p (u j) -> p u j", u=2)
                nc.vector.tensor_tensor(s2_v, sp_v,
                                        biasm_tiles[ic][:, h0:h0 + 2, :], OP.add)
                negmax2 = wpool.tile([128, 2], f32, name="negmax2", bufs=4)
                nc.vector.tensor_reduce(negmax2[:], s2_v,
                                        mybir.AxisListType.X, OP.max, negate=True)
                z2 = wpool.tile([128, 2], f32, name="z2", bufs=4)
                e_sbs = []
                for u in range(2):
                    e_sb = wpool.tile([128, N], bf16, name="e_sb", tag="e", bufs=4)
                    nc.scalar.activation(e_sb[:], s2_sb[:, u * N:(u + 1) * N],
                                         AF.Exp, bias=negmax2[:, u:u + 1],
                                         scale=1.0, accum_out=z2[:, u:u + 1])
                    e_sbs.append(e_sb)
                flush_pend()
                pend[0] = (ic, e_sbs, z2)
            flush_pend()

        def tav_stage(st, t, pm_sb):
            """Transpose pm + AV matmuls for head pair t, interleaved per
            j-chunk; each head accumulates in its own PSUM bank."""
            v_sb, ot_sb = st["v"], st["ot"]
            h0 = 2 * t
            av_u = [ps_o.tile([128, N], f32, name=f"av_ps{u}", tag="o", bufs=2)
                    for u in range(2)]
            for jc in range(4):
                tp = ps_t.tile([128, 1024], bf16, name="tp_ps", tag="tp")
                for u in range(2):
                    for ic in range(IC):
                        nc.tensor.transpose(
                            tp[:, u * N + ic * 128: u * N + ic * 128 + 128],
                            pm_sb[:, (u * IC + ic) * N + jc * 128:
                                  (u * IC + ic) * N + jc * 128 + 128],
                            idb_sb[:],
                        )
                pmt = wpool.tile([128, 1024], bf16, name="pmt", bufs=4)
                nc.scalar.copy(pmt[:], tp[:])
                for u in range(2):
                    h = h0 + u
                    nc.tensor.matmul(
                        av_u[u][u * 64:(u + 1) * 64, :],
                        lhsT=v_sb[:, jc * F + h * 64: jc * F + h * 64 + 64],
                        rhs=pmt[:, u * N:(u + 1) * N],
                        start=(jc == 0), stop=(jc == 3),
                        tile_position=(0, u * 64),
                    )
            for u in range(2):
                nc.scalar.copy(ot_sb[u * 64:(u + 1) * 64, t * N:(t + 1) * N],
                               av_u[u][u * 64:(u + 1) * 64, :])

        def final_stage(b, st):
            ot_sb = st["ot"]
            for icq in range(IC):
                f_ps = ps_mm.tile([128, N], f32, name="f_ps", tag="mm")
                for cc in range(4):
                    nc.tensor.matmul(
                        f_ps[:],
                        lhsT=ot_sb[:, cc * N + icq * 128: cc * N + icq * 128 + 128],
                        rhs=wo_sb[:, cc * F:(cc + 1) * F],
                        start=(cc == 0), stop=False,
                    )
                nc.tensor.matmul(f_ps[:], lhsT=ones_row, rhs=bo_row,
                                 start=False, stop=True)
                out_sb = wpool.tile([128, N], f32, name="out_sb", bufs=2)
                nc.scalar.copy(out_sb[:], f_ps[:])
                nc.sync.dma_start(out_d[b, icq * 128:(icq + 1) * 128, :], out_sb[:])

        # ---- cross-batch pair pipeline: 8 head-pair stages (2 batches x 4),
        # TAV of pair k-1 runs under S of pair k; batch 1's inputs/ndT are
        # prefetched mid-way through batch 0 ----
        sts = {0: batch_inputs(0)}
        ndt_stage(sts[0])
        pm = {}
        for k in range(9):
            if k < 8:
                b, t = divmod(k, 4)
                st = sts[b]
                qk_proj(st, t)
                pm[k] = spool.tile([128, 2 * IC * N], bf16, name="pm_sb",
                                   tag="pm", bufs=2)
                s_stage(b, st, t, pm[k])
                if t == 0:
                    for jc in range(4):
                        v_proj(st, jc)
            if k >= 1:
                bb, tt = divmod(k - 1, 4)
                tav_stage(sts[bb], tt, pm[k - 1])
                if tt == 3:
                    final_stage(bb, sts[bb])
            if k == 2:
                sts[1] = batch_inputs(1)
            if k == 3:
                ndt_stage(sts[1])


    return _split_multiwaits(nc) if split else nc


def make_in_maps(inputs: dict) -> list:
    inp = {k: np.asarray(v) for k, v in inputs.items()}
    ident = np.eye(128, dtype=np.float32)
    brow = np.concatenate([
        inp["bv"].astype(np.float32),
        inp["bo"].astype(np.float32),
        np.ones(128, np.float32),
    ]).reshape(1, F + F + 128)
    shared = {
        "wq": np.ascontiguousarray(inp["Wq"].astype(np.float32) * 8.0),
        "wk": np.ascontiguousarray(inp["Wk"].astype(np.float32)),
        "wv": np.ascontiguousarray(inp["Wv"].astype(np.float32)),
        "wo": np.ascontiguousarray(inp["Wo"].astype(ml_dtypes.bfloat16)),
        "bq": (inp["bq"].astype(np.float32) * 8.0).reshape(F, 1),
        "bk": inp["bk"].astype(np.float32).reshape(F, 1),
        "brow": brow,
        "idf": ident,
        "idb": ident.astype(ml_dtypes.bfloat16),
    }
    # Fold the additive mask into the bias (reference: where(mask!=0, -inf, S+bias))
    biasm = np.where(inp["attn_mask"][..., None] != 0, np.float32(-1e30),
                     inp["attn_bias"].astype(np.float32)).astype(ml_dtypes.bfloat16)
    mul_t = np.ascontiguousarray(
        inp["attn_mul"].astype(np.float32).transpose(0, 3, 1, 2)
    ).astype(ml_dtypes.bfloat16)
    in_maps = []
    for c in range(NCORES):
        sl = slice(c * BPC, (c + 1) * BPC)
        m = dict(shared)
        m["nd"] = np.ascontiguousarray(inp["ndata"][sl].astype(np.float32))
        m["bias"] = np.ascontiguousarray(biasm[sl])
        m["mul"] = np.ascontiguousarray(mul_t[sl])
        in_maps.append(m)
    return in_maps


def kernel(**inputs) -> np.ndarray:
    nc = build_nc()
    in_maps = make_in_maps(inputs)
    res = run_bass_kernel_spmd(nc, in_maps, list(range(NCORES)))
    out = np.concatenate([np.asarray(res.results[c]["out"]) for c in range(NCORES)],
                         axis=0)
    return out.astype(np.float32)


if __name__ == "__main__":
    nc = build_nc()
    print("built ok")



# revision 3
# speedup vs baseline: 1.0833x; 1.0833x over previous
"""Biased multi-head attention Trainium2 kernel (Bass/Tile), 8-way data-parallel over batch.

Reference computation (per batch b):
  q = (nd @ Wq + bq) * 8 ; k = nd @ Wk + bk ; v = nd @ Wv + bv      (8 heads, d=64)
  S[h] = Q_h K_h^T + bias[..,h] ; S[mask==1] = -inf
  A = softmax(S, -1) * mul[..,h]
  out = concat_h(A_h V_h) @ Wo + bo

Device mapping (per core, 2 batches):
  - host supplies nd pre-transposed (ndt [F, N] f32r); all fp32 matmuls run in
    float32r (full-rate PE streaming, ~1e-4 rel err)
  - QT/KT = W^T @ ndT with per-partition bias added via K=1 ones-row matmuls
  - scores: per (head-pair, i-chunk): K=64 f32r matmul + bias/mask chunk added
    in the same PSUM group via an id128 @ bias bf16 matmul
  - row max via one DVE tensor_reduce over [128, 2, 512]; exp on ACT straight
    from PSUM (bf16 out, Z via accumulator)
  - A = E*mul (gpsimd TT) then *1/Z (DVE tensor_scalar, 4x mode), bf16
  - A^T via DMA: pm chunks stored to DRAM scratch, transposed back with
    dma_start_transpose (DRAM->SBUF, exact)
  - AV accumulates per head pair in one PSUM tile (tile_position col split)
  - final projection OT bf16 @ Wo bf16 + bo ones-row matmul
"""

import os
import sys

import numpy as np

try:
    import concourse  # noqa: F401
except ImportError:
    sys.path.insert(0, "/opt/trn_rl_repo")

import ml_dtypes
from concourse import bass, mybir
from concourse.bass_utils import run_bass_kernel_spmd
from concourse.tile import TileContext

B, N, F, H, D = 16, 512, 512, 8, 64
NCORES = 8
BPC = B // NCORES  # batches per core
IC = N // 128      # 128-row chunks per sequence

f32 = mybir.dt.float32
f32r = mybir.dt.float32r
bf16 = mybir.dt.bfloat16
AF = mybir.ActivationFunctionType
OP = mybir.AluOpType
AX = mybir.AxisListType


def _split_multiwaits(nc: bass.Bass) -> bass.Bass:
    """Walrus codegen only accepts one sync-wait per ISA instruction; hoist
    extra waits into single-wait NoOps on the same engine right before."""
    for fn in nc.m.functions:
        for blk in fn.blocks:
            new = []
            for inst in blk.instructions:
                si = getattr(inst, "sync_info", None)
                ow = list(si.on_wait) if (si is not None and si.on_wait) else []
                if len(ow) > 1:
                    for j, w in enumerate(ow[:-1]):
                        new.append(mybir.InstNoOp(
                            name=f"{inst.name}-wsplit{j}",
                            engine=inst.engine,
                            ins=[], outs=[],
                            sync_info=mybir.SyncInfo(on_wait=[w], on_update=[]),
                        ))
                    si.on_wait = ow[-1:]
                    inst.sync_info = si
                new.append(inst)
            blk.instructions[:] = new
    return nc


def build_nc(split: bool = True) -> bass.Bass:
    nc = bass.Bass()

    ndt_d = nc.declare_dram_parameter("ndt", [BPC, F, N], f32r, isOutput=False)
    # bias with mask pre-merged (-1e30), layout [b, pair, ic, p, u, j] bf16
    bias_d = nc.declare_dram_parameter("bias", [BPC, 4, IC, 128, 2, N], bf16,
                                       isOutput=False)
    # attn_mul, same layout
    mul_d = nc.declare_dram_parameter("mul", [BPC, 4, IC, 128, 2, N], bf16,
                                      isOutput=False)
    wq_d = nc.declare_dram_parameter("wq", [F, F], f32r, isOutput=False)  # x8
    wk_d = nc.declare_dram_parameter("wk", [F, F], f32r, isOutput=False)
    wv_d = nc.declare_dram_parameter("wv", [F, F], f32r, isOutput=False)
    wo_d = nc.declare_dram_parameter("wo", [F, F], bf16, isOutput=False)
    # [bq*8 | bk | bv | bo | ones] as one row, f32r
    brow_d = nc.declare_dram_parameter("brow", [1, 5 * F], f32r, isOutput=False)
    idb_d = nc.declare_dram_parameter("idb", [128, 128], bf16, isOutput=False)
    out_d = nc.declare_dram_parameter("out", [BPC, N, F], f32, isOutput=True)

    with (
        TileContext(nc) as tc,
        tc.tile_pool(name="cpool", bufs=1) as cpool,
        tc.tile_pool(name="bpool", bufs=1) as bpool,
        tc.tile_pool(name="spool", bufs=2) as spool,
        tc.tile_pool(name="wpool", bufs=2) as wpool,
        tc.tile_pool(name="dpool", bufs=1, space="DRAM") as dpool,
        tc.tile_pool(name="ps_s", bufs=2, space="PSUM") as ps_s,
        tc.tile_pool(name="ps_mm", bufs=2, space="PSUM") as ps_mm,
        tc.tile_pool(name="ps_o", bufs=2, space="PSUM") as ps_o,
    ):
        # ---- constants / weights ----
        wq_sb = cpool.tile([128, 4 * F], f32r, name="wq_sb")
        wk_sb = cpool.tile([128, 4 * F], f32r, name="wk_sb")
        wv_sb = cpool.tile([128, 4 * F], f32r, name="wv_sb")
        wo_sb = cpool.tile([128, 4 * F], bf16, name="wo_sb")
        for cc in range(4):
            nc.sync.dma_start(wq_sb[:, cc * F:(cc + 1) * F], wq_d[cc * 128:(cc + 1) * 128, :])
            nc.sync.dma_start(wk_sb[:, cc * F:(cc + 1) * F], wk_d[cc * 128:(cc + 1) * 128, :])
        brow_sb = cpool.tile([1, 5 * F], f32r, name="brow_sb")
        nc.sync.dma_start(brow_sb[:], brow_d[:, :])
        bq_row = brow_sb[:, 0:F]            # [1, 512] per-ft chunks as lhsT
        bk_row = brow_sb[:, F:2 * F]
        bv_row = brow_sb[:, 2 * F:3 * F]    # rhs for V bias
        bo_row = brow_sb[:, 3 * F:4 * F]
        ones_row = brow_sb[:, 4 * F:5 * F]  # [1, 512] of ones
        idb_sb = cpool.tile([128, 128], bf16, name="idb_sb")
        nc.sync.dma_start(idb_sb[:], idb_d[:, :])

        def batch_inputs(b):
            st = {"b": b}
            ndt_sb = bpool.tile([128, 4 * N], f32r, name="ndt_sb", tag="ndt", bufs=2)
            for cc in range(4):
                nc.sync.dma_start(ndt_sb[:, cc * N:(cc + 1) * N],
                                  ndt_d[b, cc * 128:(cc + 1) * 128, :])
            st["ndt"] = ndt_sb
            st["v"] = bpool.tile([128, 4 * F], bf16, name="v_sb", tag="v", bufs=2)
            st["ot"] = bpool.tile([128, 4 * N], bf16, name="ot_sb", tag="ot", bufs=2)
            if b == 0:
                for cc in range(4):
                    nc.sync.dma_start(wv_sb[:, cc * F:(cc + 1) * F],
                                      wv_d[cc * 128:(cc + 1) * 128, :])
                    nc.sync.dma_start(wo_sb[:, cc * F:(cc + 1) * F],
                                      wo_d[cc * 128:(cc + 1) * 128, :])
            return st

        def qk_proj(st, t):
            ndt_sb = st["ndt"]
            psq = ps_mm.tile([128, 512], f32, name="ps_q", tag="mm")
            for cc in range(4):
                nc.tensor.matmul(
                    psq[:],
                    lhsT=wq_sb[:, cc * F + t * 128: cc * F + t * 128 + 128],
                    rhs=ndt_sb[:, cc * N:(cc + 1) * N],
                    start=(cc == 0), stop=False,
                )
            nc.tensor.matmul(psq[:], lhsT=bq_row[:, t * 128:(t + 1) * 128],
                             rhs=ones_row, start=False, stop=True)
            qt = wpool.tile([128, 512], f32r, name="qt", tag="qt", bufs=3)
            nc.scalar.copy(qt[:], psq[:])
            psk = ps_mm.tile([128, 512], f32, name="ps_k", tag="mm")
            for cc in range(4):
                nc.tensor.matmul(
                    psk[:],
                    lhsT=wk_sb[:, cc * F + t * 128: cc * F + t * 128 + 128],
                    rhs=ndt_sb[:, cc * N:(cc + 1) * N],
                    start=(cc == 0), stop=False,
                )
            nc.tensor.matmul(psk[:], lhsT=bk_row[:, t * 128:(t + 1) * 128],
                             rhs=ones_row, start=False, stop=True)
            kt = wpool.tile([128, 512], f32r, name="kt", tag="kt", bufs=3)
            nc.vector.tensor_copy(kt[:], psk[:])
            st["qt"], st["kt"] = qt, kt

        def v_proj(st, jc):
            ndt_sb, v_sb = st["ndt"], st["v"]
            psv = ps_mm.tile([128, 512], f32, name="ps_v", tag="mm")
            for cc in range(4):
                nc.tensor.matmul(
                    psv[:],
                    lhsT=ndt_sb[:, cc * N + jc * 128: cc * N + jc * 128 + 128],
                    rhs=wv_sb[:, cc * F:(cc + 1) * F],
                    start=(cc == 0), stop=False,
                )
            nc.tensor.matmul(psv[:], lhsT=ones_row[:, 0:128], rhs=bv_row,
                             start=False, stop=True)
            nc.scalar.copy(v_sb[:, jc * F:(jc + 1) * F], psv[:])

        def s_stage(b, st, t, scr):
            """Scores + softmax for head pair t; pm chunks land in DRAM scr.
            Normalize of chunk ic is pipelined one chunk behind the exp."""
            qt, kt = st["qt"], st["kt"]
            pend = [None]

            def flush_pend():
                if pend[0] is None:
                    return
                ic, e_u, z2, mulic = pend[0]
                rz2 = wpool.tile([128, 2], f32, name="rz2", tag="rz2", bufs=4)
                nc.vector.reciprocal(rz2[:], z2[:])
                for u in range(2):
                    pm0 = wpool.tile([128, 512], bf16, name="pm0",
                                     tag="pm0", bufs=4)
                    nc.gpsimd.tensor_tensor(pm0[:], e_u[u][:], mulic[:, u, :],
                                            OP.mult)
                    pm = wpool.tile([128, 512], bf16, name="pm", tag="pm", bufs=6)
                    nc.vector.tensor_scalar(pm[:], pm0[:], rz2[:, u:u + 1],
                                            None, OP.mult)
                    nc.sync.dma_start(
                        scr[u, ic * 128:(ic + 1) * 128, :], pm[:])
                pend[0] = None

            for ic in range(IC):
                biasic = spool.tile([128, 2, N], bf16, name="biasic",
                                    tag="biasic", bufs=5)
                nc.sync.dma_start(biasic[:], bias_d[b, t, ic])
                mulic = spool.tile([128, 2, N], bf16, name="mulic",
                                   tag="mulic", bufs=5)
                nc.sync.dma_start(mulic[:], mul_d[b, t, ic])
                sp = ps_s.tile([128, 2, N], f32, name="sp", tag="sp")
                for u in range(2):
                    nc.tensor.matmul(
                        sp[:, u, :],
                        lhsT=qt[u * 64:u * 64 + 64, ic * 128:ic * 128 + 128],
                        rhs=kt[u * 64:u * 64 + 64, :],
                        start=True, stop=False,
                    )
                    nc.tensor.matmul(
                        sp[:, u, :], lhsT=idb_sb[:], rhs=biasic[:, u, :],
                        start=False, stop=True,
                    )
                negmax = wpool.tile([128, 2], f32, name="negmax",
                                    tag="negmax", bufs=4)
                nc.vector.tensor_reduce(negmax[:], sp[:], AX.X, OP.max,
                                        negate=True)
                z2 = wpool.tile([128, 2], f32, name="z2", tag="z2", bufs=4)
                e_u = []
                for u in range(2):
                    e_sb = wpool.tile([128, N], bf16, name="e_sb", tag="e",
                                      bufs=6)
                    nc.scalar.activation(e_sb[:], sp[:, u, :], AF.Exp,
                                         bias=negmax[:, u:u + 1], scale=1.0,
                                         accum_out=z2[:, u:u + 1])
                    e_u.append(e_sb)
                flush_pend()
                pend[0] = (ic, e_u, z2, mulic)
            flush_pend()

        def tav_stage(st, t, scr):
            """DMA-transpose pm from DRAM and run AV matmuls for head pair t."""
            v_sb, ot_sb = st["v"], st["ot"]
            av = ps_o.tile([128, N], f32, name="av_ps", tag="av")
            pmts = {}
            for jc in range(4):
                for u in range(2):
                    pmt = spool.tile([128, N], bf16, name="pmt", tag="pmt",
                                     bufs=10)
                    nc.sync.dma_start_transpose(
                        pmt[:], scr[u, :, jc * 128:(jc + 1) * 128])
                    pmts[jc, u] = pmt
            for jc in range(4):
                for u in range(2):
                    h = 2 * t + u
                    nc.tensor.matmul(
                        av[u * 64:(u + 1) * 64, :],
                        lhsT=v_sb[:, jc * F + h * 64: jc * F + h * 64 + 64],
                        rhs=pmts[jc, u][:],
                        start=(jc == 0), stop=(jc == 3),
                        tile_position=(0, u * 64),
                    )
            nc.vector.tensor_copy(ot_sb[:, t * N:(t + 1) * N], av[:])

        def final_stage(b, st):
            ot_sb = st["ot"]
            for icq in range(IC):
                f_ps = ps_mm.tile([128, N], f32, name="f_ps", tag="mm")
                for cc in range(4):
                    nc.tensor.matmul(
                        f_ps[:],
                        lhsT=ot_sb[:, cc * N + icq * 128: cc * N + icq * 128 + 128],
                        rhs=wo_sb[:, cc * F:(cc + 1) * F],
                        start=(cc == 0), stop=False,
                    )
                nc.tensor.matmul(f_ps[:], lhsT=ones_row[:, 0:128], rhs=bo_row,
                                 start=False, stop=True)
                out_sb = wpool.tile([128, N], f32, name="out_sb",
                                    tag="out", bufs=2)
                nc.scalar.copy(out_sb[:], f_ps[:])
                nc.sync.dma_start(out_d[b, icq * 128:(icq + 1) * 128, :], out_sb[:])

        # ---- cross-batch pair pipeline: 8 head-pair stages (2 batches x 4);
        # TAV of pair k-1 runs under S of pair k ----
        sts = {0: batch_inputs(0)}
        scrs = {}
        for k in range(9):
            if k < 8:
                b, t = divmod(k, 4)
                st = sts[b]
                qk_proj(st, t)
            if k >= 1:
                bb, tt = divmod(k - 1, 4)
                tav_stage(sts[bb], tt, scrs[k - 1])
                if tt == 3:
                    final_stage(bb, sts[bb])
            if k < 8:
                scrs[k] = dpool.tile([2, N, N], bf16, name="scr",
                                     tag="scr", bufs=3)
                s_stage(b, st, t, scrs[k])
                if t == 0:
                    for jc in range(4):
                        v_proj(st, jc)
            if k == 2:
                sts[1] = batch_inputs(1)

    mybir.codegen_inst_isa_subclasses(nc)
    return _split_multiwaits(nc) if split else nc


def make_in_maps(inputs: dict) -> list:
    inp = {k: np.asarray(v) for k, v in inputs.items()}
    ident = np.eye(128, dtype=np.float32)
    brow = np.concatenate([
        inp["bq"].astype(np.float32) * 8.0,
        inp["bk"].astype(np.float32),
        inp["bv"].astype(np.float32),
        inp["bo"].astype(np.float32),
        np.ones(F, np.float32),
    ]).reshape(1, 5 * F)
    shared = {
        "wq": np.ascontiguousarray(inp["Wq"].astype(np.float32) * 8.0),
        "wk": np.ascontiguousarray(inp["Wk"].astype(np.float32)),
        "wv": np.ascontiguousarray(inp["Wv"].astype(np.float32)),
        "wo": np.ascontiguousarray(inp["Wo"].astype(ml_dtypes.bfloat16)),
        "brow": brow,
        "idb": ident.astype(ml_dtypes.bfloat16),
    }
    # [B, i, j, h] -> [B, pair, ic, p, u, j]
    def to_chunks(x):
        x = x.transpose(0, 3, 1, 2)                      # [B, h, i, j]
        x = x.reshape(B, 4, 2, IC, 128, N)               # [B, t, u, ic, p, j]
        return np.ascontiguousarray(x.transpose(0, 1, 3, 4, 2, 5))

    biasm = np.where(inp["attn_mask"][..., None] != 0, np.float32(-1e30),
                     inp["attn_bias"].astype(np.float32))
    biasm = to_chunks(biasm).astype(ml_dtypes.bfloat16)
    mul_t = to_chunks(inp["attn_mul"].astype(np.float32)).astype(ml_dtypes.bfloat16)
    ndt = np.ascontiguousarray(
        inp["ndata"].astype(np.float32).transpose(0, 2, 1))
    in_maps = []
    for c in range(NCORES):
        sl = slice(c * BPC, (c + 1) * BPC)
        m = dict(shared)
        m["ndt"] = np.ascontiguousarray(ndt[sl])
        m["bias"] = np.ascontiguousarray(biasm[sl])
        m["mul"] = np.ascontiguousarray(mul_t[sl])
        in_maps.append(m)
    return in_maps


def kernel(**inputs) -> np.ndarray:
    nc = build_nc()
    in_maps = make_in_maps(inputs)
    res = run_bass_kernel_spmd(nc, in_maps, list(range(NCORES)))
    out = np.concatenate([np.asarray(res.results[c]["out"]) for c in range(NCORES)],
                         axis=0)
    return out.astype(np.float32)


if __name__ == "__main__":
    nc = build_nc()
    print("built ok")
